# revision 1
# baseline (speedup 1.0000x reference)
"""Trainium2 Bass kernel for nn_AttentionOutput (complex causal leaky-relu attention).

Reference (B=4, N=4096, F=64), per batch:
    sr = (Qr@Kr^T - Qi@Ki^T)/sqrt(N); si = (Qr@Ki^T + Qi@Kr^T)/sqrt(N)
    wr = tril * leaky_relu(sr);        wi = tril * leaky_relu(si)
    out_r = (wr@Vr)@W_att^T + b;       out_i = (wi@Vi)@W_att^T + b

Distribution: 2 cores per batch.  Core parity h processes j-blocks J === h
(mod 2) for ALL 4096 query rows; causal work is then identical across cores
(slot I needs 2I+2 j-blocks), so a single SPMD program serves all 8 cores and
the host sums the two partial outputs per batch.

Host-side layout prep removes every on-device transpose:
  - scores contract over p = f*2+c (128 partitions, ONE matmul per component):
    sr = Qmodr . K^T where Qmodr = Q with odd columns negated, and
    si = Qmodi . K^T where Qmodi = Q with column pairs swapped; K stays plain.
    Both Q variants are fed pre-transposed [128, N].
  - V' = (1/64) V @ W_att^T folds the score scale and the output projection
    into the attention-value matmul (leaky_relu is positively homogeneous).
  - output is stored transposed ([128, N]: y_r^T on rows 0:64, y_i^T on
    64:128); the host untransposes, interleaves, adds bias, sums parities.

leaky_relu lowering (RELU_CORR): leaky(s) = 0.99*relu(s) + 0.01*s.  For
causally-full j-blocks the 0.01*s term telescopes into a per-slot constant
matmul: 0.01*sum_J s_J @ V' = (0.01*sum_J kp_J @ V'_J)^T-style correction
M_slot, precomputed on the host and accumulated into the y PSUM bank.  So a
full tile needs ONE PSUM-draining op (relu), split between ScalarE and
VectorE for bandwidth.  Diagonal tiles compute u = mask*s (VectorE, also
drains) and w = relu(u), feeding two matmuls against 0.99*V' and 0.01*V'.

NOTE: ACT Lrelu reading PSUM hangs TRN2 (empirically) — never emit it.
"""

import numpy as np

import concourse.bacc as bacc
import concourse.tile as tile
from concourse import mybir
from concourse.bass_utils import run_bass_kernel_spmd

B, N, F = 4, 4096, 64
P = 128             # = 2*F: score contraction width / partition count
JB = 128            # j-block width
IBW = 512           # i-block (slot) width
NSLOT = N // IBW    # 8 slots
NJPAR = N // JB // 2  # 16 parity j-blocks per core
NEG = 0.01
SCALE = 1.0 / 64.0  # 1/sqrt(N)
NCORES = 8

_DT = mybir.dt.float32
MM_BF16 = True      # bf16 matmul inputs: 4x PE throughput, half the DMA bytes
# Of the full-tile relu drains, every ACT_EVERY-th goes to VectorE, the rest
# to ScalarE (Relu activation).  0 -> all on VectorE (if ACT relu is unsafe).
ACT_SHARE = True    # ScalarE participates in full-tile relu drains
_CACHE: dict = {}


def _build_nc():
    nc = bacc.Bacc("TRN2", target_bir_lowering=False, num_devices=NCORES)
    dt = _DT
    mdt = mybir.dt.bfloat16 if MM_BF16 else _DT  # matmul input dtype
    qrT = nc.dram_tensor("qrT", [P, N], mdt, kind="ExternalInput")
    qiT = nc.dram_tensor("qiT", [P, N], mdt, kind="ExternalInput")
    kp = nc.dram_tensor("kp", [P, NJPAR * JB], mdt, kind="ExternalInput")
    # va = 0.99 * V' (relu term), vb = 0.01 * V' (raw term, diagonal only)
    var_ = nc.dram_tensor("var", [P, NJPAR * F], mdt, kind="ExternalInput")
    vai = nc.dram_tensor("vai", [P, NJPAR * F], mdt, kind="ExternalInput")
    vbr = nc.dram_tensor("vbr", [P, NJPAR * F], mdt, kind="ExternalInput")
    vbi = nc.dram_tensor("vbi", [P, NJPAR * F], mdt, kind="ExternalInput")
    # per-slot correction weights: 0.01 * sum_{full J} kp_J @ V'_J  [P, 64]
    mcr = nc.dram_tensor("mcr", [P, NSLOT * F], mdt, kind="ExternalInput")
    mci = nc.dram_tensor("mci", [P, NSLOT * F], mdt, kind="ExternalInput")
    dmask = nc.dram_tensor("dmask", [2, JB, IBW], mdt, kind="ExternalInput")
    out = nc.dram_tensor("out", [P, N], dt, kind="ExternalOutput")

    relu = mybir.ActivationFunctionType.Relu
    mul_op = mybir.AluOpType.mult
    max_op = mybir.AluOpType.max

    with tile.TileContext(nc) as tc:
        with (
            tc.tile_pool(name="res", bufs=1) as res,
            tc.tile_pool(name="wp", bufs=6) as wp,
            tc.tile_pool(name="osb", bufs=2) as osb,
            tc.tile_pool(name="spsum", bufs=5, space="PSUM") as spsum,
            tc.tile_pool(name="ypsum", bufs=1, space="PSUM") as ypsum,
        ):
            sb_qr = res.tile([P, N], mdt, tag="qr")
            sb_qi = res.tile([P, N], mdt, tag="qi")
            for c in range(8):
                sl = slice(c * 512, (c + 1) * 512)
                nc.sync.dma_start(out=sb_qr[:, sl], in_=qrT[:, sl])
                nc.sync.dma_start(out=sb_qi[:, sl], in_=qiT[:, sl])
            sb_k = res.tile([P, NJPAR * JB], mdt, tag="k")
            for c in range(4):
                sl = slice(c * 512, (c + 1) * 512)
                nc.sync.dma_start(out=sb_k[:, sl], in_=kp[:, sl])
            sb_var = res.tile([P, NJPAR * F], mdt, tag="var")
            sb_vai = res.tile([P, NJPAR * F], mdt, tag="vai")
            sb_vbr = res.tile([P, NJPAR * F], mdt, tag="vbr")
            sb_vbi = res.tile([P, NJPAR * F], mdt, tag="vbi")
            for c in range(2):
                sl = slice(c * 512, (c + 1) * 512)
                nc.sync.dma_start(out=sb_var[:, sl], in_=var_[:, sl])
                nc.sync.dma_start(out=sb_vai[:, sl], in_=vai[:, sl])
                nc.sync.dma_start(out=sb_vbr[:, sl], in_=vbr[:, sl])
                nc.sync.dma_start(out=sb_vbi[:, sl], in_=vbi[:, sl])
            sb_mcr = res.tile([P, NSLOT * F], mdt, tag="mcr")
            sb_mci = res.tile([P, NSLOT * F], mdt, tag="mci")
            nc.sync.dma_start(out=sb_mcr, in_=mcr[:])
            nc.sync.dma_start(out=sb_mci, in_=mci[:])
            sb_m0 = res.tile([JB, IBW], mdt, tag="m0")
            sb_m1 = res.tile([JB, IBW], mdt, tag="m1")
            nc.sync.dma_start(out=sb_m0, in_=dmask[0])
            nc.sync.dma_start(out=sb_m1, in_=dmask[1])
            sb_masks = (sb_m0, sb_m1)

            drain_ctr = 0  # alternates full-tile relu drains ACT/DVE
            for s in range(NSLOT):
                cnt = 2 * s + 2
                isl = slice(s * IBW, (s + 1) * IBW)
                y_r = ypsum.tile([64, IBW], dt, tag="yr")
                y_i = ypsum.tile([64, IBW], dt, tag="yi")
                for p in range(cnt):
                    ksl = slice(p * JB, (p + 1) * JB)
                    vsl = slice(p * F, (p + 1) * F)
                    s_r = spsum.tile([JB, IBW], dt, tag="s")
                    nc.tensor.matmul(s_r[:], sb_k[:, ksl], sb_qr[:, isl],
                                     start=True, stop=True)
                    s_i = spsum.tile([JB, IBW], dt, tag="s")
                    nc.tensor.matmul(s_i[:], sb_k[:, ksl], sb_qi[:, isl],
                                     start=True, stop=True)
                    first = (p == 0)
                    for s_ps, sb_va, sb_vb, y_ps in (
                            (s_r, sb_var, sb_vbr, y_r),
                            (s_i, sb_vai, sb_vbi, y_i)):
                        if p < cnt - 2:
                            # full block: w = relu(s); 0.01*s handled by mcorr
                            w = wp.tile([JB, IBW], mdt, tag="w")
                            if ACT_SHARE and drain_ctr % 3 != 2:
                                nc.scalar.activation(w[:], s_ps[:], relu)
                            else:
                                nc.vector.tensor_scalar_max(w[:], s_ps[:], 0.0)
                            drain_ctr += 1
                            nc.tensor.matmul(y_ps[:], sb_va[:, vsl], w[:],
                                             start=first, stop=False)
                        else:
                            # diagonal block: u = mask*s (drain), w = relu(u)
                            mk = sb_masks[p - (cnt - 2)]
                            u = wp.tile([JB, IBW], mdt, tag="u")
                            nc.vector.tensor_tensor(out=u[:], in0=s_ps[:],
                                                    in1=mk[:], op=mul_op)
                            nc.tensor.matmul(y_ps[:], sb_vb[:, vsl], u[:],
                                             start=first, stop=False)
                            w = wp.tile([JB, IBW], mdt, tag="w")
                            nc.vector.tensor_scalar_max(w[:], u[:], 0.0)
                            last = (s == 0 and p == cnt - 1)
                            nc.tensor.matmul(y_ps[:], sb_va[:, vsl], w[:],
                                             start=False, stop=last)
                # correction matmul: y += (0.01 * sum_full kp_J @ V'_J)^T @ q
                if s > 0:
                    msl = slice(s * F, (s + 1) * F)
                    nc.tensor.matmul(y_r[:], sb_mcr[:, msl], sb_qr[:, isl],
                                     start=False, stop=True)
                    nc.tensor.matmul(y_i[:], sb_mci[:, msl], sb_qi[:, isl],
                                     start=False, stop=True)
                # tail: accumulators to SBUF, then DMA out transposed
                y_r_sb = osb.tile([64, IBW], dt, tag="ysbr")
                y_i_sb = osb.tile([64, IBW], dt, tag="ysbi")
                nc.scalar.copy(y_r_sb[:], y_r[:])
                nc.scalar.copy(y_i_sb[:], y_i[:])
                nc.sync.dma_start(out=out[0:64, isl], in_=y_r_sb[:])
                nc.sync.dma_start(out=out[64:128, isl], in_=y_i_sb[:])
    nc.compile()
    return nc


def _prep_inputs(Q, K, V, W_att, b_att):
    """Host-side re-layout: per-core in_maps for run_bass_kernel_spmd."""
    Q = np.asarray(Q, dtype=np.float32)
    K = np.asarray(K, dtype=np.float32)
    V = np.asarray(V, dtype=np.float32)
    W_att = np.asarray(W_att, dtype=np.float32)

    Qf = Q.reshape(B, N, P)          # [b, i, f*2+c]
    Kf = K.reshape(B, N, P)
    Vpr = SCALE * (V[..., 0] @ W_att.T)   # [B, N, F]
    Vpi = SCALE * (V[..., 1] @ W_att.T)

    # causal masks for a slot's last two parity j-blocks, per core parity h:
    # diagonal sub-block d = 2k+h of the slot's group of 4
    jj = np.arange(JB)[:, None]
    ii = np.arange(IBW)[None, :]
    masks = {h: np.stack([(ii >= jj + JB * (2 * k + h)).astype(np.float32)
                          for k in range(2)]) for h in (0, 1)}

    if MM_BF16:
        import ml_dtypes
        cvt = lambda a: np.ascontiguousarray(a).astype(ml_dtypes.bfloat16)
    else:
        cvt = lambda a: np.ascontiguousarray(a, dtype=np.float32)

    in_maps = []
    for c in range(NCORES):
        b, h = divmod(c, 2)
        Qmodr = Qf[b].copy()
        Qmodr[:, 1::2] *= -1.0
        Qmodi = np.empty_like(Qf[b])
        Qmodi[:, 0::2] = Qf[b][:, 1::2]
        Qmodi[:, 1::2] = Qf[b][:, 0::2]
        # parity-packed K: [P, NJPAR*JB], position pp holds block J = 2*pp+h
        kp3 = Kf[b].reshape(N // JB, JB, P)[h::2]          # [16, j, p]
        kp = kp3.transpose(2, 0, 1).reshape(P, -1)         # [p, pp*JB+j]
        vr3 = Vpr[b].reshape(N // JB, JB, F)[h::2]         # [16, j, f]
        vi3 = Vpi[b].reshape(N // JB, JB, F)[h::2]
        vpr = vr3.transpose(1, 0, 2).reshape(JB, -1)       # [j, pp*F+f]
        vpi = vi3.transpose(1, 0, 2).reshape(JB, -1)
        # per-slot correction: 0.01 * sum over FULL blocks (pos < cnt-2 = 2s)
        # of kp_block^T-contracted V': prod[pp] = kp3[pp].T-free... [p, f]
        prod_r = np.einsum('bjp,bjf->bpf', kp3, vr3)       # [16, p, f]
        prod_i = np.einsum('bjp,bjf->bpf', kp3, vi3)
        pre_r = np.concatenate(
            [np.zeros((1, P, F), np.float32), np.cumsum(prod_r, axis=0)])
        pre_i = np.concatenate(
            [np.zeros((1, P, F), np.float32), np.cumsum(prod_i, axis=0)])
        mcr = np.concatenate([NEG * pre_r[2 * s] for s in range(NSLOT)], axis=1)
        mci = np.concatenate([NEG * pre_i[2 * s] for s in range(NSLOT)], axis=1)
        in_maps.append({
            "qrT": cvt(Qmodr.T),
            "qiT": cvt(Qmodi.T),
            "kp": cvt(kp),
            "var": cvt((1.0 - NEG) * vpr),
            "vai": cvt((1.0 - NEG) * vpi),
            "vbr": cvt(NEG * vpr),
            "vbi": cvt(NEG * vpi),
            "mcr": cvt(mcr),
            "mci": cvt(mci),
            "dmask": cvt(masks[h]),
        })
    return in_maps


def _gather(results, b_att):
    b_att = np.asarray(b_att, dtype=np.float32)
    out = np.empty((B, N, F, 2), dtype=np.float32)
    for b in range(B):
        y = results[2 * b]["out"] + results[2 * b + 1]["out"]  # [128, N]
        out[b, :, :, 0] = y[0:64].T + b_att[None, :]
        out[b, :, :, 1] = y[64:128].T + b_att[None, :]
    return out


def kernel(Q, K, V, W_att, b_att):
    if "nc" not in _CACHE:
        _CACHE["nc"] = _build_nc()
    nc = _CACHE["nc"]
    in_maps = _prep_inputs(Q, K, V, W_att, b_att)
    res = run_bass_kernel_spmd(nc, in_maps, core_ids=list(range(NCORES)))
    return _gather(res.results, b_att)

